# revision 1
# baseline (speedup 1.0000x reference)
"""Trainium2 Bass kernel for DeepInterestNetwork (DIN) — v2.

8 cores, data-parallel over batch; each core 512 rows = 16 tiles of G=32.
Host gathers embeddings and marshals two layouts; device does attention MLP,
softmax, pooling, head MLP.

Key structure (per tile of 32 rows):
  - L1: fp8e4 DoubleRow matmuls, K=128 packed as [64,2] (k-plane, q*k-plane).
    Host ships k*64 and q*k*16384 in fp8; weights are pre-scaled so the
    psum holds 16384*z. relu(z + cb) is drained with the bias pre-scaled
    (relu homogeneity): h' = relu(16384*z + 16384*cb) = 16384*h.
  - L2: w2/64 stationary -> scores' = 256*scores; exp(scores'/256) via the
    Act scale param. Scores psum packed at partition bases {0,32,64} so ONE
    exp covers all 32 rows at 66-partition occupancy.
  - softmax denominators come free from a ones-column in the pooling stack;
    normalization folds into the per-partition scale of the interest drain.
  - pooling: DVE mul (2x mode, [128, 65, 50] stride-1) + 50 accumulating
    PE matmuls folding rank-partitions -> interest [32(g), 65].
  - sigmoid via exp + DVE reciprocal (keeps one act table: exp_and_others).
  - 2-tile software pipeline: esp shuffle DMAs (chained) land 2 tiles early.
"""

import numpy as np
import sys

for p in ("/opt/trn_rl_repo", "/opt/trn_rl_repo/concourse"):
    if p not in sys.path:
        sys.path.insert(0, p)

VOCAB, E = 100000, 64
B, T = 4096, 200
NCORES = 8
BC = B // NCORES          # 512 rows per core
G = 32                    # batch rows per tile
NTILES_FULL = BC // G     # 16
NQ = 4                    # t-quarters: slot p = 32*qt + g, t = 50*qt + r
RANKS = T // NQ           # 50
NPAIR = G // 2            # 16 row pairs
SCALE = 16384.0           # k*64 (fp8) x W*256 = 16384*z


SLOT_PERM = np.arange(G)  # scores land contiguously; slots are natural order


_nc_cache = {}


def build_nc(ntiles=NTILES_FULL):
    import concourse.bacc as bacc
    import concourse.mybir as mybir
    import concourse.tile as tile

    f32 = mybir.dt.float32
    f16 = mybir.dt.float16
    f8 = mybir.dt.float8e4
    AF = mybir.ActivationFunctionType
    ALU = mybir.AluOpType
    PM = mybir.MatmulPerfMode

    nc = bacc.Bacc("TRN2", target_bir_lowering=False, debug=False)

    kqke_d = nc.dram_tensor("kqke", [ntiles, 64, T, NPAIR, 2], f8,
                            kind="ExternalInput")
    kqko_d = nc.dram_tensor("kqko", [ntiles, 128, T, NPAIR], f8,
                            kind="ExternalInput")
    stag_d = nc.dram_tensor("stag", [ntiles, 128, E + 1, RANKS], f16,
                            kind="ExternalInput")
    cb_d = nc.dram_tensor("cb", [128, ntiles * NPAIR], f32, kind="ExternalInput")
    w1dr_d = nc.dram_tensor("w1dr", [64, 2, 64], f8, kind="ExternalInput")
    w1nd_d = nc.dram_tensor("w1nd", [128, 64], f8, kind="ExternalInput")
    w2p_d = nc.dram_tensor("w2p", [128, NPAIR, G], f16, kind="ExternalInput")
    p2m_d = nc.dram_tensor("p2m", [128, G], f16, kind="ExternalInput")
    idg_d = nc.dram_tensor("idg", [G, G], f16, kind="ExternalInput")
    dw1_d = nc.dram_tensor("dw1", [E, 128], f16, kind="ExternalInput")
    db1_d = nc.dram_tensor("db1", [128, 1], f32, kind="ExternalInput")
    dw2_d = nc.dram_tensor("dw2", [128, E], f16, kind="ExternalInput")
    db2_d = nc.dram_tensor("db2", [E, 1], f32, kind="ExternalInput")
    ow_d = nc.dram_tensor("ow", [E, 1], f16, kind="ExternalInput")
    obn_d = nc.dram_tensor("obn", [1, 1], f32, kind="ExternalInput")
    out_d = nc.dram_tensor("out", [1, ntiles * G], f32, kind="ExternalOutput")

    with tile.TileContext(nc) as tc:
        with tc.tile_pool(name="consts", bufs=1) as consts, \
             tc.tile_pool(name="kqp", bufs=3) as kq_pool, \
             tc.tile_pool(name="stp", bufs=7) as st_pool, \
             tc.tile_pool(name="hp", bufs=2) as h_pool, \
             tc.tile_pool(name="esp_", bufs=4) as es_pool, \
             tc.tile_pool(name="espp", bufs=6) as esp_pool, \
             tc.tile_pool(name="tmpp", bufs=3) as tmp_pool, \
             tc.tile_pool(name="smp", bufs=4) as sm_pool, \
             tc.tile_pool(name="php", bufs=4, space="PSUM") as ph_pool, \
             tc.tile_pool(name="scp", bufs=2, space="PSUM") as sc_pool, \
             tc.tile_pool(name="pmp", bufs=2, space="PSUM") as pm_pool:

            # ---- first tile's data before the constants (prologue latency) ----
            kqe0 = kq_pool.tile([64, T, NPAIR, 2], f8, tag="kqe")
            nc.sync.dma_start(kqe0[:], kqke_d.ap()[0])
            kqo0 = kq_pool.tile([128, T, NPAIR], f8, tag="kqo")
            nc.sync.dma_start(kqo0[:], kqko_d.ap()[0])
            st0 = st_pool.tile([128, E + 1, RANKS], f16, tag="st")
            nc.sync.dma_start(st0[:], stag_d.ap()[0])

            # ---- constants ----
            w1dr = consts.tile([64, 2, 64], f8)
            nc.sync.dma_start(w1dr[:], w1dr_d.ap())
            w1nd = consts.tile([128, 64], f8)
            nc.sync.dma_start(w1nd[:], w1nd_d.ap())
            w2p = consts.tile([128, NPAIR, G], f16)
            nc.sync.dma_start(w2p[:], w2p_d.ap())
            p2m = consts.tile([128, G], f16)
            nc.sync.dma_start(p2m[:], p2m_d.ap())
            idg = consts.tile([G, G], f16)
            nc.sync.dma_start(idg[:], idg_d.ap())
            dw1 = consts.tile([E, 128], f16)
            nc.sync.dma_start(dw1[:], dw1_d.ap())
            db1 = consts.tile([128, 1], f32)
            nc.sync.dma_start(db1[:], db1_d.ap())
            dw2 = consts.tile([128, E], f16)
            nc.sync.dma_start(dw2[:], dw2_d.ap())
            db2 = consts.tile([E, 1], f32)
            nc.sync.dma_start(db2[:], db2_d.ap())
            ow = consts.tile([E, 1], f16)
            nc.sync.dma_start(ow[:], ow_d.ap())
            obn = consts.tile([1, 1], f32)
            nc.sync.dma_start(obn[:], obn_d.ap())
            cball = consts.tile([128, ntiles * NPAIR], f32)
            nc.sync.dma_start(cball[:], cb_d.ap())
            itall = consts.tile([E, ntiles * G], f16)



            pend = []  # per-tile dicts advancing through pipeline stations

            def s2a(ent):
                """station +2: pooling multiply (DVE)."""
                tmp = tmp_pool.tile([128, E + 1, RANKS], f16, tag="tmp")
                nc.vector.tensor_mul(
                    tmp[:], ent["st"][:],
                    ent["esp"][:, None, :].broadcast_to([128, E + 1, RANKS]),
                )
                ent["tmp"] = tmp

            def s2b_fold(ent, r0, r1):
                """station +3, PE part: rank-fold matmuls [r0, r1)."""
                if "pit" not in ent:
                    pit_t = pm_pool.tile([G, E + 1], f32, tag="pm")
                    ent["pit"] = pit_t
                for r in range(r0, r1):
                    nc.tensor.matmul(
                        ent["pit"][:], p2m[:], ent["tmp"][:, :, r],
                        start=(r == 0), stop=(r == RANKS - 1),
                        skip_group_check=True,
                    )

            def s2b_norm(ent):
                """station +3, DVE part: softmax normalization."""
                pit = ent["pit"]
                rd = sm_pool.tile([G, 1], f32, tag="rd")
                nc.vector.reciprocal(rd[:], pit[:, E : E + 1])
                it16 = sm_pool.tile([G, E], f16, tag="it16")
                nc.vector.tensor_scalar_mul(it16[:], pit[:, 0:E], rd[:])
                ent["it16"] = it16

            def s2c(ent):
                """station +4: transpose into the head-batch buffer."""
                itp = pm_pool.tile([E, G], f32, tag="pm")
                nc.tensor.matmul(itp[:], ent["it16"][:], idg[:])
                nc.scalar.copy(itall[:, ent["ti"] * G : (ent["ti"] + 1) * G],
                               itp[:])

            def aged(ti, age):
                for ent in pend:
                    if ti - ent["ti"] == age:
                        return ent
                return None

            def load_tile(ti):
                kqe = kq_pool.tile([64, T, NPAIR, 2], f8, tag="kqe")
                nc.sync.dma_start(kqe[:], kqke_d.ap()[ti])
                kqo = kq_pool.tile([128, T, NPAIR], f8, tag="kqo")
                nc.sync.dma_start(kqo[:], kqko_d.ap()[ti])
                st = st_pool.tile([128, E + 1, RANKS], f16, tag="st")
                nc.sync.dma_start(st[:], stag_d.ap()[ti])
                return kqe, kqo, st

            nxt = (kqe0, kqo0, st0)
            for ti in range(ntiles):
                # ---- input DMAs: prefetch one tile ahead (SP -> HWDGE) ----
                kqe, kqo, st = nxt
                if ti + 1 < ntiles:
                    nxt = load_tile(ti + 1)
                cbt = cball[:, ti * NPAIR : (ti + 1) * NPAIR]
                ent_a = aged(ti, 3)
                ent_b = aged(ti, 4)
                ent_c = aged(ti, 5)

                kqve = kqe[:].rearrange("p t j two -> p j two t")
                kqvo = kqo[:].rearrange("p t j -> p j t")
                hs = []
                phs = None
                for j in range(NPAIR):
                    if j % 2 == 0:
                        phs = ph_pool.tile([128, 2, 256], f32, tag="ph")
                    # even row: fp8 DoubleRow (dst must start at partition 0)
                    nc.tensor.matmul(
                        phs[0:64, j % 2, 0:T], w1dr[:], kqve[:, j],
                        perf_mode=PM.DoubleRow,
                    )
                    # odd row: plain fp8 K=128 matmul at column position 64
                    nc.tensor.matmul(
                        phs[64:128, j % 2, 0:T], w1nd[:], kqvo[:, j],
                    )
                    # fold matmuls of tile ti-3 fill PE stalls in the L1 phase
                    if ent_b is not None:
                        s2b_fold(ent_b, 3 * j, min(3 * j + 3, RANKS))
                    h = h_pool.tile([128, T], f16, tag=f"h{j}")
                    # both relus of a psum-bank duo go to ONE engine
                    # (BankOverlapTracker serializes cross-engine bank sharing)
                    if (j // 2) % 2 == 0:
                        nc.scalar.activation(
                            h[:], phs[:, j % 2, 0:T], AF.Relu,
                            bias=cbt[:, j : j + 1],
                        )
                    else:
                        nc.vector.tensor_scalar(
                            h[:], phs[:, j % 2, 0:T],
                            cbt[:, j : j + 1], 0.0, ALU.add, ALU.max,
                        )
                    hs.append(h)

                if ent_b is not None:
                    s2b_fold(ent_b, 48, RANKS)
                    s2b_norm(ent_b)
                if ent_c is not None:
                    s2c(ent_c)
                    pend.remove(ent_c)

                # ---- L2: all 32 score rows accumulate into sc [32, 200] ----
                sc = sc_pool.tile([G, 256], f32, tag="sc")
                for j in range(NPAIR):
                    nc.tensor.matmul(
                        sc[:, 0:T], w2p[:, j, :], hs[j][:],
                        start=(j == 0), stop=(j == NPAIR - 1),
                        skip_group_check=True,
                    )

                # ---- softmax numerators: es = exp(scores'/256) ----
                es = es_pool.tile([G, NQ, RANKS], f16, tag="es")
                es_v = es[:].rearrange("p qt r -> p (qt r)")
                nc.scalar.activation(es_v, sc[:, 0:T], AF.Exp,
                                     scale=1.0 / 256)

                # ---- esp broadcast: quarter qt of t -> partitions 32*qt+g ----
                esp = esp_pool.tile([128, RANKS], f16, tag="esp")
                stripe_eng = [nc.gpsimd, nc.gpsimd, nc.sync, nc.scalar]
                for qt in range(NQ):
                    stripe_eng[qt].dma_start(
                        esp[32 * qt : 32 * qt + 32, :], es[:, qt, :])

                # pooling multiply for tile ti-2, last on DVE this iteration
                if ent_a is not None:
                    s2a(ent_a)

                pend.append({"ti": ti, "st": st, "esp": esp})

            # epilogue: drain the stations, then the batched head MLP
            k = ntiles
            while pend:
                ent_a = aged(k, 3)
                ent_b = aged(k, 4)
                ent_c = aged(k, 5)
                if ent_b is not None:
                    s2b_fold(ent_b, 0, RANKS)
                    s2b_norm(ent_b)
                if ent_c is not None:
                    s2c(ent_c)
                    pend.remove(ent_c)
                if ent_a is not None:
                    s2a(ent_a)
                k += 1
            pd1 = pm_pool.tile([128, ntiles * G], f32, tag="pm")
            nc.tensor.matmul(pd1[:], dw1[:], itall[:])
            d1h = sm_pool.tile([128, ntiles * G], f16, tag="d1h")
            nc.scalar.activation(d1h[:], pd1[:], AF.Relu, bias=db1[:])
            pd2 = pm_pool.tile([E, ntiles * G], f32, tag="pm")
            nc.tensor.matmul(pd2[:], dw2[:], d1h[:])
            d2h = sm_pool.tile([E, ntiles * G], f16, tag="d2h")
            nc.scalar.activation(d2h[:], pd2[:], AF.Relu, bias=db2[:])
            po = pm_pool.tile([1, ntiles * G], f32, tag="pm")
            nc.tensor.matmul(po[:], ow[:], d2h[:])
            # sigmoid(x+ob) = 1/(1+exp(-x-ob))
            en = sm_pool.tile([1, ntiles * G], f32, tag="en")
            nc.scalar.activation(en[:], po[:], AF.Exp, scale=-1.0, bias=obn[:])
            sp1 = sm_pool.tile([1, ntiles * G], f32, tag="sp1")
            nc.vector.tensor_scalar_add(sp1[:], en[:], 1.0)
            outall = sm_pool.tile([1, ntiles * G], f32, tag="outall")
            nc.vector.reciprocal(outall[:], sp1[:])
            nc.sync.dma_start(out_d.ap(), outall[:])

    nc.compile()
    return nc


def marshal_inputs(query, keys, emb, att_w1, att_b1, att_w2, att_b2,
                   deep_w1, deep_b1, deep_w2, deep_b2, out_w, out_b,
                   ntiles=NTILES_FULL):
    import concourse.mybir as mybir
    f8np = mybir.dt.np(mybir.dt.float8e4)

    query = np.asarray(query).astype(np.int64)
    keys = np.asarray(keys).astype(np.int64)
    emb = np.asarray(emb, dtype=np.float32)
    a1 = np.asarray(att_w1, dtype=np.float32)
    Wq, Wk, Wd, Wm = a1[0:64], a1[64:128], a1[128:192], a1[192:256]
    b1 = np.asarray(att_b1, np.float32)
    w2 = np.asarray(att_w2, np.float32)[:, 0]

    # constants (shared across cores)
    w1dr = np.stack([(Wk - Wd) * 256.0, Wm], axis=1).astype(f8np)  # [64,2,64]
    w1nd = np.vstack([(Wk - Wd) * 256.0, Wm]).astype(f8np)         # [128,64]
    # w2p[:, j, :]: pair j -> score rows (2j, 2j+1)
    w2p = np.zeros((128, NPAIR, G), np.float16)
    for j in range(NPAIR):
        w2p[0:64, j, 2 * j] = (w2 / 64.0).astype(np.float16)
        w2p[64:128, j, 2 * j + 1] = (w2 / 64.0).astype(np.float16)
    p2m = (np.arange(128)[:, None] % G == np.arange(G)[None, :]).astype(np.float16)
    idg = np.eye(G, dtype=np.float16)
    dw1 = np.asarray(deep_w1, np.float32).astype(np.float16)
    db1 = np.asarray(deep_b1, np.float32).reshape(128, 1)
    dw2 = np.asarray(deep_w2, np.float32).astype(np.float16)
    db2 = np.asarray(deep_b2, np.float32).reshape(64, 1)
    ow = np.asarray(out_w, np.float32).astype(np.float16)
    obn = -np.asarray(out_b, np.float32).reshape(1, 1)

    rows_of_slot = np.argsort(SLOT_PERM)  # slot g' -> local row b

    in_maps = []
    for c in range(NCORES):
        rows = slice(c * BC, c * BC + ntiles * G)
        k32 = emb[keys[rows]]                      # [512, 200, 64] f32
        q32 = emb[query[rows]]                     # [512, 64] f32
        k8 = (k32 * 64.0).astype(f8np)
        qk8 = (q32[:, None, :] * k32 * SCALE).astype(f8np)
        # even rows (g=2j): DR layout [nt, 64e, 200t, 16j, 2i]
        k8r = k8.reshape(ntiles, NPAIR, 2, T, E)
        qk8r = qk8.reshape(ntiles, NPAIR, 2, T, E)
        kqke = np.ascontiguousarray(np.stack(
            [k8r[:, :, 0].transpose(0, 3, 2, 1),
             qk8r[:, :, 0].transpose(0, 3, 2, 1)], axis=-1))
        # odd rows (g=2j+1): plain layout [nt, 128=(e;e), 200t, 16j]
        kqko = np.ascontiguousarray(np.concatenate(
            [k8r[:, :, 1].transpose(0, 3, 2, 1),
             qk8r[:, :, 1].transpose(0, 3, 2, 1)], axis=1))
        # stag [nt, 128=(qt,g'), 65, 50] fp16, rows permuted to slots
        k16p = k32.reshape(ntiles, G, T, E)[:, rows_of_slot]  # [nt, g', t, e]
        k16p = k16p.reshape(ntiles, G, NQ, RANKS, E)          # t = 50*qt + r
        stag = np.empty((ntiles, 128, E + 1, RANKS), np.float16)
        stag[:, :, 0:E, :] = (
            k16p.transpose(0, 2, 1, 4, 3).reshape(ntiles, 128, E, RANKS)
        )
        stag[:, :, E, :] = 1.0
        stag = np.ascontiguousarray(stag)
        # cb [128=(par*64+n), nt*16j] f32, pre-scaled by SCALE
        cbr = (q32 @ (Wq + Wd) + b1) * SCALE                  # [512, 64]
        cbr = cbr.reshape(ntiles, NPAIR, 2, E).transpose(2, 3, 0, 1)
        cb = np.ascontiguousarray(
            cbr.reshape(128, ntiles * NPAIR)).astype(np.float32)
        in_maps.append({
            "kqke": kqke, "kqko": kqko, "stag": stag, "cb": cb,
            "w1dr": w1dr, "w1nd": w1nd, "w2p": w2p, "p2m": p2m, "idg": idg,
            "dw1": dw1, "db1": db1, "dw2": dw2, "db2": db2,
            "ow": ow, "obn": obn,
        })
    return in_maps


def unpermute(res_flat, ntiles=NTILES_FULL):
    """res [1, nt*32] per core in slot order -> natural row order."""
    r = np.asarray(res_flat).reshape(ntiles, G)
    return r[:, SLOT_PERM].reshape(-1)


def kernel(**inputs) -> np.ndarray:
    from concourse.bass_utils import run_bass_kernel_spmd

    if "full" not in _nc_cache:
        _nc_cache["full"] = build_nc(NTILES_FULL)
    nc = _nc_cache["full"]
    in_maps = marshal_inputs(**inputs)
    res = run_bass_kernel_spmd(nc, in_maps, core_ids=list(range(NCORES)))
    outs = [unpermute(res.results[c]["out"]) for c in range(NCORES)]
    return np.concatenate(outs).reshape(B, 1).astype(np.float32)


if __name__ == "__main__":
    sys.path.insert(0, "/root/problem")
    import reference
    inputs = {k: np.asarray(v) for k, v in reference.setup_inputs().items()}
    expected = np.asarray(reference.reference(**inputs))
    actual = kernel(**inputs)
    err = np.abs(actual - expected).max() / (np.abs(expected).max() + 1e-12)
    print("Relative error:", err)



# revision 2
# speedup vs baseline: 3.0938x; 3.0938x over previous
"""Trainium2 Bass kernel for DeepInterestNetwork (DIN) — v3.

8 cores, data-parallel over batch; each core 512 rows = 8 tiles of G=64.
Host gathers embeddings and folds the query into per-row L1 weights; device
does the attention MLP, softmax, pooling, and head MLP.

Structure (per tile of 64 rows):
  - L1: h = relu(k @ W_r + cb) with per-row W_r = (Wk-Wd) + q_r*Wm and the
    bias folded in as a 65th contraction row (moving row = 64.0, stationary
    row = 256*cb).  Per row one fp8 matmul: stationary wrt[:, g, :] [65, 64]
    (LDWEIGHTS overlaps), moving k8 [65, 200] contiguous -> psum 16384*h.
    4 rows share one psum bank [128, 2, 200]; ONE pure-relu drain each.
  - L2: per pair p stationary w2p[:, p, :] (w2/64 at rows 2p, 2p+1), moving
    h [128, 200] f16 contiguous, 32 matmuls accumulate sc [64, 200] =
    256*scores.
  - softmax: one exp (scale 1/256) with accum_out giving the denominators;
    esn = es * recip(den) (DVE) -> weights normalized BEFORE pooling.
  - pooling: esn striped to 128 partitions (2 DMAs), DVE multiply
    tmp = stag * esn (2x mode), DVE reduce over r (4x mode), one PE fold
    matmul (stationary red, moving p2m) -> interest^T [64e, 64g] directly.
  - head MLP batched over all 512 rows at the end; sigmoid via exp + recip.
  - software pipeline with fixed lags: L1(i) | L2/exp/esn(i-1) |
    mul/red/fold(i-2) | itall copy(i-3); inputs prefetched 2 tiles ahead.
"""

import numpy as np
import sys

for p in ("/opt/trn_rl_repo", "/opt/trn_rl_repo/concourse"):
    if p not in sys.path:
        sys.path.insert(0, p)

VOCAB, E = 100000, 64
B, T = 4096, 200
NCORES = 8
BC = B // NCORES          # 512 rows per core
G = 64                    # batch rows per tile
NTILES_FULL = BC // G     # 8
NQ = 2                    # t-halves: partition p = 64*qt + g, t = 100*qt + r
RANKS = T // NQ           # 100
NPAIR = G // 2            # 32 row pairs
NBLK = G // 4             # 16 psum blocks (4 rows each)
SCALE = 16384.0           # k*64 (fp8) x W*256 (fp8) = 16384*z

_nc_cache = {}


def build_nc(ntiles=NTILES_FULL):
    import concourse.bacc as bacc
    import concourse.mybir as mybir
    import concourse.tile as tile

    f32 = mybir.dt.float32
    f16 = mybir.dt.float16
    f8 = mybir.dt.float8e4
    AF = mybir.ActivationFunctionType
    AX = mybir.AxisListType
    ALU = mybir.AluOpType

    nc = bacc.Bacc("TRN2", target_bir_lowering=False, debug=False)

    k8x_d = nc.dram_tensor("k8x", [ntiles, E + 1, G, T], f8,
                           kind="ExternalInput")
    wrt_d = nc.dram_tensor("wrt", [ntiles, E + 1, G, E], f8,
                           kind="ExternalInput")
    stag_d = nc.dram_tensor("stag", [ntiles, 128, E, RANKS], f16,
                            kind="ExternalInput")
    w2p_d = nc.dram_tensor("w2p", [128, NPAIR, G], f16, kind="ExternalInput")
    p2m_d = nc.dram_tensor("p2m", [128, G], f16, kind="ExternalInput")
    dw1_d = nc.dram_tensor("dw1", [E, 128], f16, kind="ExternalInput")
    db1_d = nc.dram_tensor("db1", [128, 1], f32, kind="ExternalInput")
    dw2_d = nc.dram_tensor("dw2", [128, E], f16, kind="ExternalInput")
    db2_d = nc.dram_tensor("db2", [E, 1], f32, kind="ExternalInput")
    ow_d = nc.dram_tensor("ow", [E, 1], f16, kind="ExternalInput")
    obn_d = nc.dram_tensor("obn", [1, 1], f32, kind="ExternalInput")
    out_d = nc.dram_tensor("out", [1, ntiles * G], f32, kind="ExternalOutput")

    with tile.TileContext(nc) as tc:
        with tc.tile_pool(name="consts", bufs=1) as consts, \
             tc.tile_pool(name="kqp", bufs=3) as kq_pool, \
             tc.tile_pool(name="wrp", bufs=3) as wr_pool, \
             tc.tile_pool(name="stp", bufs=5) as st_pool, \
             tc.tile_pool(name="hp", bufs=2) as h_pool, \
             tc.tile_pool(name="tmpp", bufs=2) as tmp_pool, \
             tc.tile_pool(name="smp", bufs=4) as sm_pool, \
             tc.tile_pool(name="espp", bufs=3) as esp_pool, \
             tc.tile_pool(name="redp", bufs=2) as red_pool, \
             tc.tile_pool(name="php", bufs=3, space="PSUM") as ph_pool, \
             tc.tile_pool(name="scp", bufs=2, space="PSUM") as sc_pool, \
             tc.tile_pool(name="pmp", bufs=2, space="PSUM") as pm_pool:

            # ---- first tile's L1 data before the constants ----
            def load_tile(ti):
                k8 = kq_pool.tile([E + 1, G, T], f8, tag="k8")
                nc.sync.dma_start(k8[:], k8x_d.ap()[ti])
                wr = wr_pool.tile([E + 1, G, E], f8, tag="wr")
                nc.sync.dma_start(wr[:], wrt_d.ap()[ti])
                st = st_pool.tile([128, E, RANKS], f16, tag="st")
                nc.sync.dma_start(st[:], stag_d.ap()[ti])
                return {"k8": k8, "wr": wr, "st": st}

            loads = {0: load_tile(0)}

            # ---- constants ----
            w2p = consts.tile([128, NPAIR, G], f16)
            nc.sync.dma_start(w2p[:], w2p_d.ap())
            p2m = consts.tile([128, G], f16)
            nc.sync.dma_start(p2m[:], p2m_d.ap())
            dw1 = consts.tile([E, 128], f16)
            nc.sync.dma_start(dw1[:], dw1_d.ap())
            db1 = consts.tile([128, 1], f32)
            nc.sync.dma_start(db1[:], db1_d.ap())
            dw2 = consts.tile([128, E], f16)
            nc.sync.dma_start(dw2[:], dw2_d.ap())
            db2 = consts.tile([E, 1], f32)
            nc.sync.dma_start(db2[:], db2_d.ap())
            ow = consts.tile([E, 1], f16)
            nc.sync.dma_start(ow[:], ow_d.ap())
            obn = consts.tile([1, 1], f32)
            nc.sync.dma_start(obn[:], obn_d.ap())
            itall = consts.tile([E, ntiles * G], f16)

            loads[1] = load_tile(1)

            stations = {}  # ti -> dict of live tiles per pipeline station

            for i in range(ntiles + 3):
                ti_l1 = i
                ti_l2 = i - 1
                ti_pool = i - 2
                ti_copy = i - 3

                # prefetch tile i+2
                if i + 2 < ntiles:
                    loads[i + 2] = load_tile(i + 2)

                # ---- itall copy (ti_copy) ----
                if 0 <= ti_copy < ntiles:
                    ent = stations.pop(ti_copy)
                    nc.scalar.copy(
                        itall[:, ti_copy * G : (ti_copy + 1) * G], ent["pit"][:]
                    )

                # ---- L2 + exp + esn + esp (ti_l2) ----
                if 0 <= ti_l2 < ntiles:
                    ent = stations[ti_l2]
                    hall = ent["hall"]
                    sc = sc_pool.tile([G, T], f32, tag="sc")
                    for m in range(NBLK):
                        for s in range(2):
                            p = 2 * m + s
                            nc.tensor.matmul(
                                sc[:], w2p[:, p, :], hall[:, m, s, :],
                                start=(p == 0), stop=(p == NPAIR - 1),
                                skip_group_check=True,
                            )
                    es = sm_pool.tile([G, NQ, RANKS], f16, tag="es")
                    den = sm_pool.tile([G, 1], f32, tag="den")
                    es_v = es[:].rearrange("p q r -> p (q r)")
                    nc.scalar.activation(es_v, sc[:], AF.Exp,
                                         scale=1.0 / 256, accum_out=den[:])
                    rd = sm_pool.tile([G, 1], f32, tag="rd")
                    nc.vector.reciprocal(rd[:], den[:])
                    esn = sm_pool.tile([G, NQ, RANKS], f16, tag="esn")
                    nc.vector.tensor_scalar_mul(esn[:], es[:], rd[:])
                    esp = esp_pool.tile([128, RANKS], f16, tag="esp")
                    nc.gpsimd.dma_start(esp[0:G, :], esn[:, 0, :])
                    nc.scalar.dma_start(esp[G : 2 * G, :], esn[:, 1, :])
                    ent["esp"] = esp

                # ---- pooling multiply + reduce (ti_pool) on DVE ----
                if 0 <= ti_pool < ntiles:
                    ent = stations[ti_pool]
                    tmp = tmp_pool.tile([128, E, RANKS], f16, tag="tmp")
                    nc.vector.tensor_mul(
                        tmp[:], ent["st"][:],
                        ent["esp"][:, None, :].broadcast_to([128, E, RANKS]),
                    )
                    red = red_pool.tile([128, E], f16, tag="red")
                    with nc.allow_low_precision("pooled weights sum to 1"):
                        nc.vector.tensor_reduce(red[:], tmp[:], AX.X, ALU.add)
                    ent["red"] = red

                # ---- L1 (ti_l1): 16 blocks x 4 rows ----
                if ti_l1 < ntiles:
                    ld = loads[ti_l1]
                    k8, wr = ld["k8"], ld["wr"]
                    hall = h_pool.tile([128, NBLK, 2, T], f16, tag="hall")
                    for m in range(NBLK):
                        ph = ph_pool.tile([128, 2, T], f32, tag="ph")
                        for s in range(2):
                            for par in range(2):
                                g = 4 * m + 2 * s + par
                                nc.tensor.matmul(
                                    ph[64 * par : 64 * par + 64, s, :],
                                    wr[:, g, :], k8[:, g, :],
                                )
                        nc.scalar.activation(hall[:, m, :, :], ph[:], AF.Relu)
                    stations[ti_l1] = {"hall": hall, "st": ld["st"]}
                    del loads[ti_l1]

                # ---- fold matmul (ti_pool): interest^T into psum ----
                if 0 <= ti_pool < ntiles:
                    ent = stations[ti_pool]
                    pit = pm_pool.tile([E, G], f32, tag="pm")
                    nc.tensor.matmul(pit[:], ent["red"][:], p2m[:])
                    ent["pit"] = pit

            # ---- epilogue: batched head MLP over all 512 rows ----
            pd1 = pm_pool.tile([128, ntiles * G], f32, tag="pm")
            nc.tensor.matmul(pd1[:], dw1[:], itall[:])
            d1h = sm_pool.tile([128, ntiles * G], f16, tag="d1h")
            nc.scalar.activation(d1h[:], pd1[:], AF.Relu, bias=db1[:])
            pd2 = pm_pool.tile([E, ntiles * G], f32, tag="pm")
            nc.tensor.matmul(pd2[:], dw2[:], d1h[:])
            d2h = sm_pool.tile([E, ntiles * G], f16, tag="d2h")
            nc.scalar.activation(d2h[:], pd2[:], AF.Relu, bias=db2[:])
            po = pm_pool.tile([1, ntiles * G], f32, tag="pm")
            nc.tensor.matmul(po[:], ow[:], d2h[:])
            # sigmoid(x+ob) = 1/(1+exp(-x-ob))
            en = sm_pool.tile([1, ntiles * G], f32, tag="en")
            nc.scalar.activation(en[:], po[:], AF.Exp, scale=-1.0, bias=obn[:])
            sp1 = sm_pool.tile([1, ntiles * G], f32, tag="sp1")
            nc.vector.tensor_scalar_add(sp1[:], en[:], 1.0)
            outall = sm_pool.tile([1, ntiles * G], f32, tag="outall")
            nc.vector.reciprocal(outall[:], sp1[:])
            nc.sync.dma_start(out_d.ap(), outall[:])

    nc.compile()
    return nc


def marshal_inputs(query, keys, emb, att_w1, att_b1, att_w2, att_b2,
                   deep_w1, deep_b1, deep_w2, deep_b2, out_w, out_b,
                   ntiles=NTILES_FULL):
    import concourse.mybir as mybir
    f8np = mybir.dt.np(mybir.dt.float8e4)

    query = np.asarray(query).astype(np.int64)
    keys = np.asarray(keys).astype(np.int64)
    emb = np.asarray(emb, dtype=np.float32)
    a1 = np.asarray(att_w1, dtype=np.float32)
    Wq, Wk, Wd, Wm = a1[0:64], a1[64:128], a1[128:192], a1[192:256]
    Wkd = Wk - Wd
    Wqd = Wq + Wd
    b1 = np.asarray(att_b1, np.float32)
    w2 = np.asarray(att_w2, np.float32)[:, 0]

    # constants (shared across cores)
    w2p = np.zeros((128, NPAIR, G), np.float16)
    for p in range(NPAIR):
        w2p[0:64, p, 2 * p] = (w2 / 64.0).astype(np.float16)
        w2p[64:128, p, 2 * p + 1] = (w2 / 64.0).astype(np.float16)
    p2m = (np.arange(128)[:, None] % G == np.arange(G)[None, :]).astype(np.float16)
    dw1 = np.asarray(deep_w1, np.float32).astype(np.float16)
    db1 = np.asarray(deep_b1, np.float32).reshape(128, 1)
    dw2 = np.asarray(deep_w2, np.float32).astype(np.float16)
    db2 = np.asarray(deep_b2, np.float32).reshape(64, 1)
    ow = np.asarray(out_w, np.float32).astype(np.float16)
    obn = -np.asarray(out_b, np.float32).reshape(1, 1)

    in_maps = []
    for c in range(NCORES):
        rows = slice(c * BC, c * BC + ntiles * G)
        k32 = emb[keys[rows]]                      # [512, 200, 64] f32
        q32 = emb[query[rows]]                     # [512, 64] f32

        k32r = k32.reshape(ntiles, G, T, E)
        # k8x [nt, 65e, g, t]: 64*k, bias row = 64.0
        k8x = np.empty((ntiles, E + 1, G, T), np.float32)
        k8x[:, 0:E] = k32r.transpose(0, 3, 1, 2) * 64.0
        k8x[:, E] = 64.0
        k8x = k8x.astype(f8np)

        # wrt [nt, 65e, g, h]: 256*((Wk-Wd) + q_g*Wm); bias row 256*cb
        Wr = (Wkd[None, :, :] + q32[:, :, None] * Wm[None, :, :]) * 256.0
        cb = (q32 @ Wqd + b1) * 256.0              # [512, 64]
        wrt = np.empty((ntiles, E + 1, G, E), np.float32)
        wrt[:, 0:E] = Wr.reshape(ntiles, G, E, E).transpose(0, 2, 1, 3)
        wrt[:, E] = cb.reshape(ntiles, G, E)
        wrt = wrt.astype(f8np)

        # stag [nt, 128=(qt,g), e, r]: k  (t = 100*qt + r)
        stag = np.ascontiguousarray(
            k32r.reshape(ntiles, G, NQ, RANKS, E)
            .transpose(0, 2, 1, 4, 3)
            .reshape(ntiles, 128, E, RANKS)
        ).astype(np.float16)

        in_maps.append({
            "k8x": k8x, "wrt": wrt, "stag": stag,
            "w2p": w2p, "p2m": p2m,
            "dw1": dw1, "db1": db1, "dw2": dw2, "db2": db2,
            "ow": ow, "obn": obn,
        })
    return in_maps


def kernel(**inputs) -> np.ndarray:
    from concourse.bass_utils import run_bass_kernel_spmd

    if "full" not in _nc_cache:
        _nc_cache["full"] = build_nc(NTILES_FULL)
    nc = _nc_cache["full"]
    in_maps = marshal_inputs(**inputs)
    res = run_bass_kernel_spmd(nc, in_maps, core_ids=list(range(NCORES)))
    outs = [np.asarray(res.results[c]["out"]).reshape(-1) for c in range(NCORES)]
    return np.concatenate(outs).reshape(B, 1).astype(np.float32)


if __name__ == "__main__":
    sys.path.insert(0, "/root/problem")
    import reference
    inputs = {k: np.asarray(v) for k, v in reference.setup_inputs().items()}
    expected = np.asarray(reference.reference(**inputs))
    actual = kernel(**inputs)
    err = np.abs(actual - expected).max() / (np.abs(expected).max() + 1e-12)
    print("Relative error:", err)
